# revision 12
# baseline (speedup 1.0000x reference)
"""DifferentiatedTransformerBlock on 8 TRN2 NeuronCores.

Sharding: core c = (batch b=c//2, sequence-half j=c%2). Each core computes
both streams for its 1024-token half of its batch. Cross-stream K/V
dependencies (full-sequence a2 after the causal action self-attn, full s3
after the a2s cross-attn) are satisfied with two 2-rank AllGathers across
the (b,0)/(b,1) pair; K/V projections over the full sequence are
recomputed per core from the gathered raw streams.

Layout: all activations feature-major [D, tokens]. LayerNorm is folded
into the matmuls: out = (x*rstd) @ (g.*W) + [mr;1] @ [-colsum(g.*W); bias']
with bias' = b_ln@W + b. Per-token rstd rows are broadcast across
partitions with K=1 PE matmuls. Attention (scores/AV) runs in bf16 with
denominators obtained from an interleaved ones-column in the V projection;
projections run in fp32r (fp32 bits fed to the PE's truncated-multiply
path) with fp32 PSUM accumulation.
"""
import os
import sys

for _p in ("/opt/trn_rl_repo", os.path.expanduser("~/.axon_site/_ro/trn_rl_repo")):
    if os.path.isdir(_p) and _p not in sys.path:
        sys.path.insert(0, _p)

import numpy as np
import ml_dtypes
import concourse.bass as bass
import concourse.tile as tile
from concourse import mybir, bacc
from concourse.bass import ds
from concourse.bass_utils import run_bass_kernel_spmd

F32 = mybir.dt.float32
F32R = mybir.dt.float32r
BF16 = mybir.dt.bfloat16
AF = mybir.ActivationFunctionType

P = 128
D = 1024
KC = D // P          # 8 feature chunks
S = 2048             # full sequence
T = 1024             # tokens per core
H = 16
DH = 64
NP = H // 2          # 8 head pairs
FF = 4096
EPS = 1e-5
N_CORES = 8
RG = [list(range(8))]  # 8-rank AllGather (2-rank shared-out unsupported)


def _fmv(ap):
    """[D, T] DRAM AP -> [P, KC, T] chunked feature-major view."""
    return ap.rearrange("(ko ki) t -> ki ko t", ki=P)


class _Ctx:
    pass


def _build():
    nc = bacc.Bacc("TRN2", target_bir_lowering=False, debug=False,
                   num_devices=N_CORES)
    g = _Ctx()
    g.nc = nc

    # ---------------- external I/O ----------------
    ei = {}
    def din(name, shape, dt=F32):
        ei[name] = nc.dram_tensor(name, shape, dt, kind="ExternalInput")
        return ei[name]

    din("xs_full", [D, S]); din("xa_full", [D, S])
    din("xs_own", [D, T]); din("xa_own", [D, T])
    for m in ("m1", "m2", "m3", "m4"):
        din(f"{m}_wq", [D + 2, D]); din(f"{m}_wk", [D + 2, D])
        din(f"{m}_wv", [D + 2, H * (DH + 1)]); din(f"{m}_wo", [D + 2, D])
    din("w1_s", [D + 2, FF]); din("w2_s", [FF + 2, D])
    din("w1_a", [D + 2, FF]); din("w2_a", [FF + 2, D])
    din("w2b_s", [D, 1]); din("w2b_a", [D, 1])
    din("cmask", [S, T], BF16)
    din("cones", [P, S])
    din("cones_bf", [2, S], BF16)
    s_out = nc.dram_tensor("s_out", [D, T], F32, kind="ExternalOutput")
    a_out = nc.dram_tensor("a_out", [D, T], F32, kind="ExternalOutput")

    tc_cm = tile.TileContext(nc)
    tc = tc_cm.__enter__()
    lp = nc.allow_low_precision(reason="float32r rounding of matmul operands")
    lp.__enter__()

    # ---------------- pools ----------------
    po = _Ctx()
    ctxmgrs = []
    def pool(name, bufs, space="SBUF"):
        cm = tc.tile_pool(name=name, bufs=bufs, space=space)
        ctxmgrs.append(cm)
        return cm.__enter__()

    po.io = pool("io", 3)          # stream tiles [P,512]
    po.ot = pool("ot", 2)          # evac out tiles
    po.rb = pool("rb", 2)          # residual base tiles
    po.sq = pool("sq", 2)          # squares + mlp hidden
    po.rows = pool("rows", 1)      # row stats (explicit bufs per tile)
    po.rbc = pool("rbc", 1)        # rstd broadcast tiles
    po.xsr = pool("xsr", 1)        # scaled-x staging [P,KC,512]
    po.wres = pool("wres", 1)      # resident weight
    po.kfm = pool("kfm", 1)        # K feature-major / mlp xs3
    po.vtm = pool("vtm", 1)        # V token-major / mlp out accum
    po.qfm = pool("qfm", 1)        # Q per-range
    po.ctx = pool("ctx", 1)        # attention context
    po.pb = pool("pb", 3)          # P tiles
    po.mk = pool("mk", 1)          # mask tiles
    po.rdb = pool("rdb", 1)        # denom broadcast sbuf
    po.cons = pool("cons", 1)
    po.dram = pool("dram", 1, space="DRAM")
    po.ps = pool("ps", 3, space="PSUM")      # pw [P,512]
    po.psav = pool("psav", 2, space="PSUM")  # AV accumulators
    po.psr = pool("psr", 2, space="PSUM")    # stat rows
    po.psb = pool("psb", 1, space="PSUM")    # broadcasts

    # ---------------- constants ----------------
    cones = ei["cones"].ap()
    ones_col = po.cons.tile([P, 1], F32R, name="ones_col")
    nc.sync.dma_start(out=ones_col, in_=cones[:, 0:1].bitcast(F32R))
    ones_row = po.cons.tile([1, P], F32R, name="ones_row")
    nc.sync.dma_start(out=ones_row, in_=cones[0:1, 0:P].bitcast(F32R))
    ones_bf = po.cons.tile([1, P], BF16, name="ones_bf")
    nc.sync.dma_start(out=ones_bf, in_=ei["cones_bf"].ap()[0:1, 0:P])
    eps_t = po.cons.tile([1, 1], F32, name="eps_t")
    nc.vector.memset(eps_t, EPS)

    # ---------------- DRAM scratch ----------------
    sres = po.dram.tile([D, T], F32, name="sres", bufs=2)    # s1
    ares = po.dram.tile([D, T], F32, name="ares", bufs=2)    # a4
    cc_a2_in = po.dram.tile([D, T], F32, name="cc_a2_in")
    cc_a2_out = po.dram.tile([N_CORES * D, T], F32, name="cc_a2_out",
                             addr_space="Shared")
    cc_s3_in = po.dram.tile([D, T], F32, name="cc_s3_in")
    cc_s3_out = po.dram.tile([N_CORES * D, T], F32, name="cc_s3_out",
                             addr_space="Shared")
    pid = nc.sync.partition_id()
    blk = (pid // 2) * (2 * D)  # row offset of this pair's gathered blocks

    def dram_src(ap2d):
        v = _fmv(ap2d)
        return lambda k, sl: v[:, k, sl]

    def gathered_src(cc):
        def src(k, sl):
            h, st = divmod(sl.start, T)
            w = sl.stop - sl.start
            return cc[ds(blk + h * D + k * P, P), slice(st, st + w)]
        return src

    # ---------------- LN stats ----------------
    def ln_stats(src, Ttok, tag, rbc_full=None, spill=None):
        """Per-token LN stats. Fills aug [2,Ttok] (row0=mr, row1=1) and
        either a persistent rbc_full [P,Ttok] or, with spill (a [P,KC,S]
        view), scales+spills each 512-range with a transient rbc chunk."""
        aug = po.rows.tile([2, Ttok], F32R, name=f"aug_{tag}",
                           tag="augf" if Ttok == S else "augo", bufs=1)
        nc.sync.dma_start(out=aug, in_=cones[0:2, 0:Ttok].bitcast(F32R))
        for r in range(Ttok // 512):
            sl = slice(r * 512, r * 512 + 512)
            psum = po.psr.tile([1, 512], F32, name="st_sum", tag="psr")
            psq = po.psr.tile([1, 512], F32, name="st_sq", tag="psr")
            for k in range(KC):
                xt = po.io.tile([P, 512], F32R, name="st_xt", tag="io")
                nc.sync.dma_start(out=xt, in_=src(k, sl).bitcast(F32R))
                nc.tensor.matmul(psum[:], lhsT=ones_col[:], rhs=xt[:],
                                 start=(k == 0), stop=(k == KC - 1))
                sqt = po.sq.tile([P, 512], F32R, name="st_sq2", tag="sqs",
                                 bufs=1)
                nc.vector.tensor_mul(sqt[:], xt[:].bitcast(F32),
                                     xt[:].bitcast(F32))
                nc.tensor.matmul(psq[:], lhsT=ones_col[:], rhs=sqt[:],
                                 start=(k == 0), stop=(k == KC - 1))
            mean = po.rows.tile([1, 512], F32, name="mean", tag="sc", bufs=3)
            nc.scalar.activation(out=mean[:], in_=psum[:], func=AF.Copy,
                                 scale=1.0 / D)
            var = po.rows.tile([1, 512], F32, name="var", tag="sc", bufs=3)
            nc.scalar.activation(out=var[:], in_=psq[:], func=AF.Copy,
                                 scale=1.0 / D)
            m2 = po.rows.tile([1, 512], F32, name="m2", tag="sc", bufs=3)
            nc.vector.tensor_mul(m2[:], mean[:], mean[:])
            nc.vector.tensor_sub(var[:], var[:], m2[:])
            nc.scalar.activation(out=var[:], in_=var[:], func=AF.Sqrt,
                                 bias=eps_t[:], scale=1.0)
            rstd_r = po.rows.tile([1, 512], F32R, name="rstd_r", tag="sc",
                                  bufs=3)
            nc.vector.reciprocal(out=rstd_r[:], in_=var[:])
            nc.vector.tensor_mul(aug[0:1, sl], mean[:],
                                 rstd_r[:].bitcast(F32))
            pbc = po.psb.tile([P, 512], F32, name="pbc", tag="psb")
            nc.tensor.matmul(pbc[:], lhsT=ones_row[:], rhs=rstd_r[:],
                             start=True, stop=True)
            if rbc_full is not None:
                nc.scalar.activation(out=rbc_full[:, sl], in_=pbc[:],
                                     func=AF.Copy)
            else:
                rbc_r = po.rbc.tile([P, 512], F32R, name="rbc_r", tag="rbcr",
                                    bufs=2)
                nc.scalar.activation(out=rbc_r[:], in_=pbc[:], func=AF.Copy)
                for k in range(KC):
                    xt = po.io.tile([P, 512], F32R, name="sp_xt", tag="io")
                    nc.sync.dma_start(out=xt, in_=src(k, sl).bitcast(F32R))
                    sp = po.ot.tile([P, 512], F32, name="sp", tag="ot")
                    nc.vector.tensor_mul(sp[:], xt[:].bitcast(F32),
                                         rbc_r[:].bitcast(F32))
                    nc.sync.dma_start(out=spill[:, k, sl], in_=sp[:])
        return aug

    def stage_xs(src, rbc, sl):
        """Scaled-x staging tile [P,KC,512] F32R for token slice sl."""
        st = po.xsr.tile([P, KC, 512], F32R, name="xstage", tag="xsr")
        for k in range(KC):
            xt = po.io.tile([P, 512], F32R, name="sg_xt", tag="io")
            nc.sync.dma_start(out=xt, in_=src(k, sl).bitcast(F32R))
            nc.vector.tensor_mul(st[:, k, :], xt[:].bitcast(F32),
                                 rbc[:, sl].bitcast(F32))
        return st

    def stage_raw(src, sl):
        st = po.xsr.tile([P, KC, 512], F32R, name="xstage", tag="xsr")
        for k in range(KC):
            nc.sync.dma_start(out=st[:, k, :], in_=src(k, sl).bitcast(F32R))
        return st

    def load_w(name, ncols):
        ap = ei[name].ap()
        wt = po.wres.tile([P, KC, ncols], F32R, name=f"wt_{name}", tag="wres")
        nc.sync.dma_start(
            out=wt, in_=ap[0:D, :].rearrange("(ko ki) n -> ki ko n",
                                             ki=P).bitcast(F32R))
        wa = po.sq.tile([2, ncols], F32R, name=f"wa_{name}", tag="sq",
                         bufs=1)
        nc.sync.dma_start(out=wa, in_=ap[D:D + 2, :].bitcast(F32R))
        return wt, wa

    # ---------------- attention module ----------------
    def attn(mod, q_src, kv_src, res_src, out_ap, causal, v_from_xs):
        """One attention module; writes residual-updated output (fm) to
        out_ap ([D, T] DRAM)."""
        xsp = po.dram.tile([D, S], F32, name=f"xsp_{mod}", tag="xsp", bufs=2)
        xspv = _fmv(xsp[:])
        aug_f = ln_stats(kv_src, S, f"{mod}f", spill=xspv)
        xs_src = dram_src(xsp[:])

        # K projection -> kfm [P, NP, S] bf16
        kfm = po.kfm.tile([P, NP, S], BF16, name=f"kfm_{mod}", tag="kfm")
        wk, wka = load_w(f"{mod}_wk", D)
        for r in range(4):
            sl = slice(r * 512, r * 512 + 512)
            st = stage_raw(xs_src, sl)
            for p in range(NP):
                ps = po.ps.tile([P, 512], F32, name="kp", tag="pw")
                for k in range(KC):
                    nc.tensor.matmul(ps[:], lhsT=wk[:, k, p * P:(p + 1) * P],
                                     rhs=st[:, k, :], start=(k == 0),
                                     stop=False)
                nc.tensor.matmul(ps[:], lhsT=wka[:, p * P:(p + 1) * P],
                                 rhs=aug_f[:, sl], start=False, stop=True)
                nc.vector.tensor_copy(kfm[:, p, sl], ps[:])

        # V projection (token-major, interleaved ones cols) -> vtm bf16
        NV = H * (DH + 1)  # 1040
        vtm = po.vtm.tile([P, S // P, NV], BF16, name=f"vtm_{mod}", tag="vtm")
        wv, wva = load_w(f"{mod}_wv", NV)
        vslices = [slice(0, 512), slice(512, 1024), slice(1024, NV)]
        for r in range(4):
            sl = slice(r * 512, r * 512 + 512)
            st = (stage_raw(xs_src, sl) if v_from_xs
                  else stage_raw(kv_src, sl))
            for tt in range(4):
                t_abs = 4 * r + tt
                tsl = slice(tt * P, (tt + 1) * P)
                for nsl in vslices:
                    w = nsl.stop - nsl.start
                    ps = po.ps.tile([P, 512], F32, name="vp", tag="pw")
                    for k in range(KC):
                        nc.tensor.matmul(ps[:, :w], lhsT=st[:, k, tsl],
                                         rhs=wv[:, k, nsl], start=(k == 0),
                                         stop=False)
                    nc.tensor.matmul(
                        ps[:, :w],
                        lhsT=aug_f[:, slice(r * 512 + tt * P,
                                            r * 512 + (tt + 1) * P)],
                        rhs=wva[:, nsl], start=False, stop=True)
                    nc.vector.tensor_copy(vtm[:, t_abs, nsl], ps[:, :w])

        # Q-side stats on own half
        rbc_o = po.rbc.tile([P, T], F32R, name=f"rbco_{mod}", tag="rbco",
                            bufs=1)
        aug_o = ln_stats(q_src, T, f"{mod}o", rbc_full=rbc_o)
        wq, wqa = load_w(f"{mod}_wq", D)
        cm_v = ei["cmask"].ap().rearrange("(to ti) t -> ti to t", ti=P)
        qfm = po.qfm.tile([P, NP, T], BF16, name="qfm", tag="qfm")
        for r2 in range(2):
            sl = slice(r2 * 512, r2 * 512 + 512)
            st = stage_xs(q_src, rbc_o, sl)
            for p in range(NP):
                ps = po.ps.tile([P, 512], F32, name="qp", tag="pw")
                for k in range(KC):
                    nc.tensor.matmul(ps[:], lhsT=wq[:, k, p * P:(p + 1) * P],
                                     rhs=st[:, k, :], start=(k == 0),
                                     stop=False)
                nc.tensor.matmul(ps[:], lhsT=wqa[:, p * P:(p + 1) * P],
                                 rhs=aug_o[:, sl], start=False, stop=True)
                nc.vector.tensor_copy(qfm[:, p, sl], ps[:])
        wo, woa = load_w(f"{mod}_wo", D)

        for r2 in range(2):
            sl = slice(r2 * 512, r2 * 512 + 512)
            # attention per head pair
            ctxt = po.ctx.tile([P, KC, 512], F32R, name="ctxt", tag="ctx")
            for p in range(NP):
                av0 = po.psav.tile([DH + 1, 512], F32, name="av0", tag="pav")
                av1 = po.psav.tile([DH + 1, 512], F32, name="av1", tag="pav")
                for t in range(S // P):
                    sc0 = po.ps.tile([P, 512], F32, name="sc0", tag="pw")
                    sc1 = po.ps.tile([P, 512], F32, name="sc1", tag="pw")
                    tsl = slice(t * P, (t + 1) * P)
                    nc.tensor.matmul(sc0[:], lhsT=kfm[0:DH, p, tsl],
                                     rhs=qfm[0:DH, p, sl], start=True,
                                     stop=True, tile_position=(0, 0))
                    nc.tensor.matmul(sc1[:], lhsT=kfm[DH:P, p, tsl],
                                     rhs=qfm[DH:P, p, sl], start=True,
                                     stop=True, tile_position=(DH, 0))
                    p0 = po.pb.tile([P, 512], BF16, name="p0", tag="pb")
                    p1 = po.pb.tile([P, 512], BF16, name="p1", tag="pb")
                    nc.scalar.activation(out=p0[:], in_=sc0[:], func=AF.Exp)
                    nc.scalar.activation(out=p1[:], in_=sc1[:], func=AF.Exp)
                    if causal:
                        mk = po.mk.tile([P, 512], BF16, name="mk", tag="mk")
                        nc.sync.dma_start(out=mk, in_=cm_v[:, t, sl])
                        nc.vector.tensor_mul(p0[:], p0[:], mk[:])
                        nc.vector.tensor_mul(p1[:], p1[:], mk[:])
                    vsl0 = slice((2 * p) * (DH + 1), (2 * p + 1) * (DH + 1))
                    vsl1 = slice((2 * p + 1) * (DH + 1),
                                 (2 * p + 2) * (DH + 1))
                    nc.tensor.matmul(av0[:], lhsT=vtm[:, t, vsl0], rhs=p0[:],
                                     start=(t == 0), stop=(t == S // P - 1))
                    nc.tensor.matmul(av1[:], lhsT=vtm[:, t, vsl1], rhs=p1[:],
                                     start=(t == 0), stop=(t == S // P - 1))
                # normalize: ctx_h = av[0:DH] * (1/av[DH])
                rd0 = po.rows.tile([1, 512], BF16, name="rd0", tag="rd",
                                   bufs=2)
                rd1 = po.rows.tile([1, 512], BF16, name="rd1", tag="rd",
                                   bufs=2)
                nc.vector.reciprocal(out=rd0[:], in_=av0[DH:DH + 1, :])
                nc.vector.reciprocal(out=rd1[:], in_=av1[DH:DH + 1, :])
                pbc = po.psb.tile([P, 512], F32, name="avbc", tag="psb")
                nc.tensor.matmul(pbc[0:DH, :], lhsT=ones_bf[:, 0:DH],
                                 rhs=rd0[:], start=True, stop=True)
                nc.tensor.matmul(pbc[DH:P, :], lhsT=ones_bf[:, 0:DH],
                                 rhs=rd1[:], start=True, stop=True,
                                 tile_position=(0, DH))
                rdbc = po.rdb.tile([P, 512], F32, name="rdbc", tag="rdb")
                nc.scalar.activation(out=rdbc[:], in_=pbc[:], func=AF.Copy)
                nc.vector.tensor_mul(ctxt[0:DH, p, :], av0[0:DH, :],
                                     rdbc[0:DH, :])
                nc.vector.tensor_mul(ctxt[DH:P, p, :], av1[0:DH, :],
                                     rdbc[DH:P, :])

            # output projection + residual
            ov = _fmv(out_ap)
            for n in range(KC):
                ps = po.ps.tile([P, 512], F32, name="op", tag="pw")
                for k in range(KC):
                    nc.tensor.matmul(ps[:], lhsT=wo[:, k, n * P:(n + 1) * P],
                                     rhs=ctxt[:, k, :], start=(k == 0),
                                     stop=False)
                nc.tensor.matmul(ps[:], lhsT=woa[:, n * P:(n + 1) * P],
                                 rhs=aug_o[:, sl], start=False, stop=True)
                rb = po.rb.tile([P, 512], F32, name="rb", tag="rb")
                nc.sync.dma_start(out=rb, in_=res_src(n, sl))
                ot = po.ot.tile([P, 512], F32, name="ot", tag="ot")
                nc.vector.tensor_add(ot[:], ps[:], rb[:])
                nc.sync.dma_start(out=ov[:, n, sl], in_=ot[:])

    # ---------------- MLP module ----------------
    def mlp(w1n, w2n, in_src, res_src, out_ap):
        rbc_o = po.rbc.tile([P, T], F32R, name=f"rbco_{w1n}", tag="rbco",
                            bufs=1)
        aug_o = ln_stats(in_src, T, f"mlp{w1n}", rbc_full=rbc_o)
        # resident scaled input [P, KC, T]
        xs3 = po.kfm.tile([P, KC, T], F32R, name=f"xs3_{w1n}", tag="kfm")
        for r2 in range(2):
            sl = slice(r2 * 512, r2 * 512 + 512)
            for k in range(KC):
                xt = po.io.tile([P, 512], F32R, name="ml_xt", tag="io")
                nc.sync.dma_start(out=xt, in_=in_src(k, sl).bitcast(F32R))
                nc.vector.tensor_mul(xs3[:, k, sl], xt[:].bitcast(F32),
                                     rbc_o[:, sl].bitcast(F32))
        osb = po.vtm.tile([P, KC, T], F32, name=f"osb_{w1n}", tag="vtm")
        w1ap = ei[w1n].ap()
        w2ap = ei[w2n].ap()
        b2n = "w2b_s" if w2n == "w2_s" else "w2b_a"
        b2 = po.cons.tile([P, KC, 1], F32, name=f"b2_{w1n}", tag="b2",
                          bufs=1)
        nc.sync.dma_start(
            out=b2, in_=ei[b2n].ap().rearrange("(ko ki) o -> ki ko o", ki=P))
        NG = FF // 512
        for gi in range(NG):
            gsl = slice(gi * 512, gi * 512 + 512)
            w1g = po.wres.tile([P, KC, 512], F32R, name="w1g", tag="wres")
            nc.sync.dma_start(
                out=w1g, in_=w1ap[0:D, gsl].rearrange(
                    "(ko ki) n -> ki ko n", ki=P).bitcast(F32R))
            w1ga = po.cons.tile([2, 512], F32R, name="w1ga", tag="w1ga",
                                bufs=1)
            nc.sync.dma_start(out=w1ga, in_=w1ap[D:D + 2, gsl].bitcast(F32R))
            w2g = po.xsr.tile([P, 4, D], F32R, name="w2g", tag="xsr")
            nc.sync.dma_start(
                out=w2g, in_=w2ap[gsl, :].rearrange(
                    "(ko ki) n -> ki ko n", ki=P).bitcast(F32R))
            for r2 in range(2):
                sl = slice(r2 * 512, r2 * 512 + 512)
                hfm = po.sq.tile([P, 4, 512], F32R, name="hfm", tag="sq",
                                 bufs=1)
                for ht in range(4):
                    ps = po.ps.tile([P, 512], F32, name="f1", tag="pw")
                    for k in range(KC):
                        nc.tensor.matmul(
                            ps[:], lhsT=w1g[:, k, ht * P:(ht + 1) * P],
                            rhs=xs3[:, k, sl], start=(k == 0), stop=False)
                    nc.tensor.matmul(ps[:], lhsT=w1ga[:, ht * P:(ht + 1) * P],
                                     rhs=aug_o[:, sl], start=False, stop=True)
                    nc.scalar.activation(out=hfm[:, ht, :], in_=ps[:],
                                         func=AF.Gelu)
                for n in range(KC):
                    ps = po.ps.tile([P, 512], F32, name="f2", tag="pw")
                    for hk in range(4):
                        nc.tensor.matmul(ps[:],
                                         lhsT=w2g[:, hk, n * P:(n + 1) * P],
                                         rhs=hfm[:, hk, :], start=(hk == 0),
                                         stop=(hk == 3))
                    if gi == 0:
                        nc.vector.tensor_copy(osb[:, n, sl], ps[:])
                    else:
                        nc.vector.tensor_add(osb[:, n, sl], ps[:],
                                             osb[:, n, sl])
        ov = _fmv(out_ap)
        for n in range(KC):
            for r2 in range(2):
                sl = slice(r2 * 512, r2 * 512 + 512)
                rb = po.rb.tile([P, 512], F32, name="rbm", tag="rb")
                nc.sync.dma_start(out=rb, in_=res_src(n, sl))
                ot = po.ot.tile([P, 512], F32, name="otm", tag="ot")
                nc.vector.tensor_scalar(ot[:], in0=osb[:, n, sl],
                                        scalar1=b2[:, n, :], scalar2=None,
                                        op0=mybir.AluOpType.add)
                nc.vector.tensor_add(ot[:], ot[:], rb[:])
                nc.sync.dma_start(out=ov[:, n, sl], in_=ot[:])

    # ---------------- program ----------------
    src_xa_full = dram_src(ei["xa_full"].ap())
    src_xa_own = dram_src(ei["xa_own"].ap())
    src_xs_full = dram_src(ei["xs_full"].ap())
    src_xs_own = dram_src(ei["xs_own"].ap())
    src_s1 = dram_src(sres[:])
    src_a2 = dram_src(cc_a2_in[:])
    src_s3 = dram_src(cc_s3_in[:])
    src_a4 = dram_src(ares[:])
    src_a2_full = gathered_src(cc_a2_out[:])
    src_s3_full = gathered_src(cc_s3_out[:])

    # module 2: action causal self-attn -> a2 (cc_a2_in)
    attn("m2", src_xa_own, src_xa_full, src_xa_own, cc_a2_in[:],
         causal=True, v_from_xs=True)
    nc.gpsimd.collective_compute(
        "AllGather", mybir.AluOpType.bypass,
        ins=[cc_a2_in.opt()], outs=[cc_a2_out.opt()], replica_groups=RG)
    # module 1: state self-attn -> s1 (sres)
    attn("m1", src_xs_own, src_xs_full, src_xs_own, sres[:],
         causal=False, v_from_xs=True)
    # module 3: a2s cross-attn (q=s1, kv=a2 full) -> s3 (cc_s3_in)
    attn("m3", src_s1, src_a2_full, src_s1, cc_s3_in[:],
         causal=False, v_from_xs=False)
    nc.gpsimd.collective_compute(
        "AllGather", mybir.AluOpType.bypass,
        ins=[cc_s3_in.opt()], outs=[cc_s3_out.opt()], replica_groups=RG)
    # module 5: state MLP -> s_out
    mlp("w1_s", "w2_s", src_s3, src_s3, s_out.ap())
    # module 4: s2a cross-attn (q=a2, kv=s3 full) -> a4 (ares)
    attn("m4", src_a2, src_s3_full, src_a2, ares[:],
         causal=False, v_from_xs=False)
    # module 6: action MLP -> a_out
    mlp("w1_a", "w2_a", src_a4, src_a4, a_out.ap())

    for cm in reversed(ctxmgrs):
        cm.__exit__(None, None, None)
    lp.__exit__(None, None, None)
    tc_cm.__exit__(None, None, None)
    nc.compile()
    return nc


_PROG = None
_last_in_maps = None


def _get_prog():
    global _PROG
    if _PROG is None:
        _PROG = _build()
    return _PROG


# ---------------- host-side weight prep ----------------
def _stack_ln(W, b, g, bln, scale=1.0):
    W = np.asarray(W, np.float32); b = np.asarray(b, np.float32)
    g = np.asarray(g, np.float32); bln = np.asarray(bln, np.float32)
    Wp = g[:, None] * W
    return (np.concatenate([Wp, -Wp.sum(0, keepdims=True),
                            (bln @ W + b)[None]], 0) * scale).astype(np.float32)


def _stack_plain(W, b):
    W = np.asarray(W, np.float32); b = np.asarray(b, np.float32)
    z = np.zeros((1, W.shape[1]), np.float32)
    return np.concatenate([W, z, b[None]], 0).astype(np.float32)


def _interleave_v(stack):
    """[D+2, D] -> [D+2, H*(DH+1)] with a ones-producing col per head."""
    cols = []
    onec = np.zeros((stack.shape[0], 1), np.float32)
    onec[-1, 0] = 1.0
    for h in range(H):
        cols.append(stack[:, h * DH:(h + 1) * DH])
        cols.append(onec)
    return np.ascontiguousarray(np.concatenate(cols, 1))


def kernel(state_embeds, action_embeds, params):
    nc = _get_prog()
    state_embeds = np.asarray(state_embeds, np.float32)
    action_embeds = np.asarray(action_embeds, np.float32)
    p = {k: {k2: {k3: np.asarray(v3, np.float32) for k3, v3 in v2.items()}
             if isinstance(v2, dict) else np.asarray(v2, np.float32)
             for k2, v2 in v.items()} for k, v in params.items()}

    sn1g, sn1b = p["state_norm1"]["g"], p["state_norm1"]["b"]
    sn2g, sn2b = p["state_norm2"]["g"], p["state_norm2"]["b"]
    an1g, an1b = p["action_norm1"]["g"], p["action_norm1"]["b"]
    an2g, an2b = p["action_norm2"]["g"], p["action_norm2"]["b"]
    qs = 1.0 / np.sqrt(DH)

    def attw(mod, ap_, qg, qb, kg, kb, v_ln):
        d = {}
        d[f"{mod}_wq"] = _stack_ln(ap_["q"]["W"], ap_["q"]["b"], qg, qb, qs)
        d[f"{mod}_wk"] = _stack_ln(ap_["k"]["W"], ap_["k"]["b"], kg, kb)
        if v_ln is not None:
            vs = _stack_ln(ap_["v"]["W"], ap_["v"]["b"], v_ln[0], v_ln[1])
        else:
            vs = _stack_plain(ap_["v"]["W"], ap_["v"]["b"])
        d[f"{mod}_wv"] = _interleave_v(vs)
        d[f"{mod}_wo"] = _stack_plain(ap_["out"]["W"], ap_["out"]["b"])
        return d

    wmaps = {}
    wmaps.update(attw("m1", p["state_attn"], sn1g, sn1b, sn1g, sn1b,
                      (sn1g, sn1b)))
    wmaps.update(attw("m2", p["action_attn"], an1g, an1b, an1g, an1b,
                      (an1g, an1b)))
    wmaps.update(attw("m3", p["a2s_attn"], sn2g, sn2b, an2g, an2b, None))
    wmaps.update(attw("m4", p["s2a_attn"], an2g, an2b, sn2g, sn2b, None))
    wmaps["w1_s"] = _stack_ln(p["state_mlp"]["fc1"]["W"],
                              p["state_mlp"]["fc1"]["b"], sn2g, sn2b)
    wmaps["w2_s"] = _stack_plain(p["state_mlp"]["fc2"]["W"],
                                 p["state_mlp"]["fc2"]["b"])
    wmaps["w1_a"] = _stack_ln(p["action_mlp"]["fc1"]["W"],
                              p["action_mlp"]["fc1"]["b"], an2g, an2b)
    wmaps["w2_a"] = _stack_plain(p["action_mlp"]["fc2"]["W"],
                                 p["action_mlp"]["fc2"]["b"])
    wmaps["w2b_s"] = np.ascontiguousarray(
        np.asarray(p["state_mlp"]["fc2"]["b"], np.float32)[:, None])
    wmaps["w2b_a"] = np.ascontiguousarray(
        np.asarray(p["action_mlp"]["fc2"]["b"], np.float32)[:, None])
    cones = np.ones((P, S), np.float32)
    cones_bf = np.ones((2, S), ml_dtypes.bfloat16)

    in_maps = []
    for c in range(N_CORES):
        b, j = divmod(c, 2)
        sf = np.ascontiguousarray(state_embeds[b].T)
        af = np.ascontiguousarray(action_embeds[b].T)
        tk = np.arange(S)[:, None]
        tq = j * T + np.arange(T)[None, :]
        cmask = (tk <= tq).astype(ml_dtypes.bfloat16)
        m = {"xs_full": sf, "xa_full": af,
             "xs_own": np.ascontiguousarray(sf[:, j * T:(j + 1) * T]),
             "xa_own": np.ascontiguousarray(af[:, j * T:(j + 1) * T]),
             "cmask": cmask, "cones": cones, "cones_bf": cones_bf}
        m.update(wmaps)
        in_maps.append(m)

    global _last_in_maps
    _last_in_maps = in_maps
    res = run_bass_kernel_spmd(nc, in_maps, list(range(N_CORES)))
    s = np.empty((4, S, D), np.float32)
    a = np.empty((4, S, D), np.float32)
    for c in range(N_CORES):
        b, j = divmod(c, 2)
        s[b, j * T:(j + 1) * T] = res.results[c]["s_out"].T
        a[b, j * T:(j + 1) * T] = res.results[c]["a_out"].T
    return (s, a)
